# revision 11
# baseline (speedup 1.0000x reference)
"""Distributed single-head attention on 8 TRN2 NeuronCores.

Math (matches the reference):
    q = z @ Wq; k = z @ Wk; v = z @ Wv
    out = softmax(q k^T) * DK**-0.5 @ v

Sharding: z rows split 8 ways. Each core projects its own shard; K^T
(fp16, two seq-halves) and V (bf16, 6MB+2MB split) shards are
all-gathered; the collective stream (NRT comm-init barrier + 4 RDH
gathers) is the critical resource, so everything is ordered to keep it
busy from ~teens-of-us on: input DMAs start immediately (issued before
the PE warmup so no SBUF WAR dependency delays them), K-projection runs
t-outer across 6 PSUM banks so the matmuls track the input DMAs without
idle gaps, and the K^T bounce writes complete ~30us in.

Flash-style row-block attention per core:
    S^T_j = K^T[:, j-tile] ^T-matmul Q^T           (fp16 operands, f32 PSUM)
    P_j   = exp(S^T_j - 40)                        (bf16, shift-invariant)
    rowsumT = ones^T @ P                           (PE, interleaved)
    AV: j-outer into 8 PSUM banks (po[r,h] += P^T_j V_j) so late V
    blocks pace the loop without stalling it; scaled on eviction.

A warm-fill matmul block covers the PE idle window between Q-proj and
the K^T gather landing (idle >3.4us re-throttles the PE to 1.2GHz and
it takes ~10us of running to ramp back).

Precision: fp16 z/W/Q/K + f32 PSUM keeps logits to ~1e-2 abs err;
exp/V/AV in bf16. End-to-end rel err ~3e-3 (vs f32 reference).
"""

import numpy as np

SEQ, D, DK, DV = 4096, 1024, 1024, 1024
NCORES = 8
ROWS = SEQ // NCORES            # 512 rows per core
DT = D // 128                   # 8 contraction tiles (input dim)
MT = DK // 128                  # 8 dk tiles
ST = ROWS // 128                # 4 local seq tiles
JT = SEQ // 128                 # 32 global seq tiles
SHIFT = 40.0                    # constant logit shift (softmax-invariant)
SCALE = DK ** -0.5

KT_ELEMS = DK * ROWS            # fp16 K^T shard elems in packed bounce


def _build():
    import concourse.mybir as mybir
    import concourse.tile as tile
    from concourse import bacc

    F32 = mybir.dt.float32
    F16 = mybir.dt.float16
    BF16 = mybir.dt.bfloat16
    Exp = mybir.ActivationFunctionType.Exp
    Copy = mybir.ActivationFunctionType.Copy

    nc = bacc.Bacc("TRN2", target_bir_lowering=False, debug=False, num_devices=NCORES)
    d_zT = nc.declare_dram_parameter("zT", [D, ROWS], F16, isOutput=False)
    d_wq = nc.declare_dram_parameter("Wq", [D, DK], F16, isOutput=False)
    d_wk = nc.declare_dram_parameter("Wk", [D, DK], F16, isOutput=False)
    d_wv = nc.declare_dram_parameter("Wv", [D, DV], F16, isOutput=False)
    d_out = nc.declare_dram_parameter("out", [ROWS, DV], F32, isOutput=True)

    groups = [list(range(NCORES))]

    with tile.TileContext(nc) as tc:
        with (
            tc.tile_pool(name="dram", bufs=1, space="DRAM") as dram,
            tc.tile_pool(name="qt", bufs=1) as qt_pool,
            tc.tile_pool(name="misc", bufs=1) as misc,
            tc.tile_pool(name="stage", bufs=4) as stage,
            tc.tile_pool(name="outp", bufs=8) as outp,
        ):
            # ---- collective bounce buffers
            KT_H = KT_ELEMS // 2
            kt1_in = dram.tile([KT_H], BF16)
            kt1_out = dram.tile([NCORES * KT_H], BF16, addr_space="Shared")
            kt2_in = dram.tile([KT_H], BF16)
            kt2_out = dram.tile([NCORES * KT_H], BF16, addr_space="Shared")
            V1_ELEMS = 3 * 128 * DV          # s-tiles 0..2 of the V shard
            V2_ELEMS = 1 * 128 * DV          # s-tile 3
            v1_in = dram.tile([V1_ELEMS], BF16)
            v1_out = dram.tile([NCORES * V1_ELEMS], BF16, addr_space="Shared")
            v2_in = dram.tile([V2_ELEMS], BF16)
            v2_out = dram.tile([NCORES * V2_ELEMS], BF16, addr_space="Shared")

            with (
                tc.tile_pool(name="wz", bufs=1) as wz,
                tc.tile_pool(name="ps_vq", bufs=2, space="PSUM") as ps_vq,
            ):
                # Inputs first, before any SBUF consumer, so the DMA queues
                # start moving at t~10us (engine init) with no WAR waits.
                # zT+Wk interleaved on sync (K proj gates the K^T gather),
                # Wv then Wq on scalar.
                zv = d_zT.rearrange("(g t p) n -> p g t n", p=128, t=4)
                ztA = wz.tile([128, 4, ROWS], F16, name="ztA")
                ztB = wz.tile([128, 4, ROWS], F16, name="ztB")

                def wview(d_w):
                    return d_w.rearrange("(g t p) m -> p g t m", p=128, t=2)

                wkv, wvv, wqv = wview(d_wk), wview(d_wv), wview(d_wq)
                wk_sb = [wz.tile([128, 2, DK], F16, name=f"wk{g}")
                         for g in range(4)]
                wv_sb = [wz.tile([128, 2, DK], F16, name=f"wv{g}")
                        for g in range(4)]
                wq_sb = [wz.tile([128, 2, DK], F16, name=f"wq{g}")
                        for g in range(4)]
                nc.sync.dma_start(ztA[:], zv[:, 0, :, :])
                nc.sync.dma_start(wk_sb[0][:], wkv[:, 0, :, :])
                nc.sync.dma_start(wk_sb[1][:], wkv[:, 1, :, :])
                nc.sync.dma_start(ztB[:], zv[:, 1, :, :])
                nc.sync.dma_start(wk_sb[2][:], wkv[:, 2, :, :])
                nc.sync.dma_start(wk_sb[3][:], wkv[:, 3, :, :])
                for g in range(4):
                    nc.scalar.dma_start(wv_sb[g][:], wvv[:, g, :, :])
                for g in range(4):
                    nc.scalar.dma_start(wq_sb[g][:], wqv[:, g, :, :])

                ones_sb = misc.tile([128, 1], BF16)
                nc.vector.memset(ones_sb[:], 1.0)
                ones128 = misc.tile([128, 128], BF16)
                nc.vector.memset(ones128[:], 1.0)
                bias_sb = misc.tile([128, 1], F32)
                nc.vector.memset(bias_sb[:], -SHIFT)
                # touch Exp once so the ACT table set loads during proj
                warm_sb = misc.tile([128, 1], F32)
                nc.scalar.activation(warm_sb[:], ones_sb[:], Exp,
                                     bias=bias_sb[:], scale=1.0)

                # Short PE warmup bridging engine-init to the first
                # DMA-fed proj matmul.
                with (
                    tc.tile_pool(name="warmmm", bufs=1) as warm_pool,
                    tc.tile_pool(name="ps_warm", bufs=1, space="PSUM") as psw,
                ):
                    wsrc = warm_pool.tile([128, 512], BF16)
                    nc.vector.memset(wsrc[:], 0.0)
                    wps = psw.tile([128, 512], F32)
                    for _ in range(10):
                        nc.tensor.matmul(wps[:], wsrc[:, 0:128], wsrc[:],
                                         start=True, stop=True)

                def zt_ap(t):
                    return (ztA if t < 4 else ztB)[:, t % 4, :]

                def w_ap(tiles, t):
                    return tiles[t // 2][:, t % 2, :]

                # ---- K proj, t-outer across 6 banks (m 0..5), then m 6..7.
                # t-outer keeps the PE streaming at the pace of the input
                # DMAs instead of stalling a full m-chain on the last chunk.
                HN = ROWS // 2
                ktv1 = kt1_in[:].rearrange("(m p n) -> p m n", p=128, n=HN)
                ktv2 = kt2_in[:].rearrange("(m p n) -> p m n", p=128, n=HN)

                def kt_out_m(m, pk):
                    kt_stage = stage.tile([128, ROWS], F16, tag="ktstage")
                    nc.vector.tensor_copy(kt_stage[:], pk[:])
                    nc.sync.dma_start(ktv1[:, m, :],
                                      kt_stage[:, 0:HN].bitcast(BF16))
                    nc.sync.dma_start(ktv2[:, m, :],
                                      kt_stage[:, HN:ROWS].bitcast(BF16))

                with tc.tile_pool(name="ps_k", bufs=1, space="PSUM") as ps_k:
                    pk6 = [ps_k.tile([128, 512], F32, name=f"pk{i}")
                           for i in range(6)]
                    for t in range(DT):
                        for m in range(6):
                            nc.tensor.matmul(
                                pk6[m][:],
                                w_ap(wk_sb, t)[:, m * 128:(m + 1) * 128],
                                zt_ap(t),
                                start=(t == 0), stop=(t == DT - 1))
                    for m in range(6):
                        kt_out_m(m, pk6[m])
                    pk2 = [ps_k.tile([128, 512], F32, name=f"pk{i}")
                           for i in range(2)]
                    for t in range(DT):
                        for i, m in enumerate((6, 7)):
                            nc.tensor.matmul(
                                pk2[i][:],
                                w_ap(wk_sb, t)[:, m * 128:(m + 1) * 128],
                                zt_ap(t),
                                start=(t == 0), stop=(t == DT - 1))
                    for i, m in enumerate((6, 7)):
                        kt_out_m(m, pk2[i])

                nc.gpsimd.collective_compute(
                    "AllGather", mybir.AluOpType.bypass,
                    replica_groups=groups,
                    ins=[kt1_in[:].opt()], outs=[kt1_out[:].opt()])
                nc.gpsimd.collective_compute(
                    "AllGather", mybir.AluOpType.bypass,
                    replica_groups=groups,
                    ins=[kt2_in[:].opt()], outs=[kt2_out[:].opt()])

                # V shard: [ROWS, DV] bf16 -> v1_in (s 0..2) + v2_in (s 3)
                v1v = v1_in[:].rearrange("(s p m) -> p s m", p=128, m=DV)
                v2v = v2_in[:].rearrange("(s p m) -> p s m", p=128, m=DV)
                for s in range(ST):
                    for h in range(2):
                        pv = ps_vq.tile([128, 512], F32, tag="psvq")
                        for t in range(DT):
                            nc.tensor.matmul(
                                pv[:], zt_ap(t)[:, s * 128:(s + 1) * 128],
                                w_ap(wv_sb, t)[:, h * 512:(h + 1) * 512],
                                start=(t == 0), stop=(t == DT - 1))
                        v_stage = stage.tile([128, 512], BF16, tag="vstage")
                        nc.vector.tensor_copy(v_stage[:], pv[:])
                        if s < 3:
                            nc.scalar.dma_start(
                                v1v[:, s, h * 512:(h + 1) * 512], v_stage[:])
                        else:
                            nc.scalar.dma_start(
                                v2v[:, 0, h * 512:(h + 1) * 512], v_stage[:])
                nc.gpsimd.collective_compute(
                    "AllGather", mybir.AluOpType.bypass,
                    replica_groups=groups,
                    ins=[v1_in[:].opt()], outs=[v1_out[:].opt()])
                nc.gpsimd.collective_compute(
                    "AllGather", mybir.AluOpType.bypass,
                    replica_groups=groups,
                    ins=[v2_in[:].opt()], outs=[v2_out[:].opt()])

                # Q^T: [DK, ROWS] fp16, resident (overlaps the collectives)
                qt_sb = qt_pool.tile([128, MT, ROWS], F16)
                for m in range(MT):
                    pq = ps_vq.tile([128, 512], F32, tag="psvq")
                    for t in range(DT):
                        nc.tensor.matmul(pq[:],
                                         w_ap(wq_sb, t)[:, m * 128:(m + 1) * 128],
                                         zt_ap(t),
                                         start=(t == 0), stop=(t == DT - 1))
                    nc.vector.tensor_copy(qt_sb[:, m, :], pq[:])

            # Warm-fill: keep the PE clocked up through the K^T-gather wait
            # (~Qproj end to kt1 landing). Garbage math on real tiles.
            with tc.tile_pool(name="ps_wf", bufs=1, space="PSUM") as ps_wf:
                wfps = ps_wf.tile([128, 512], F32)
                for _ in range(40):
                    nc.tensor.matmul(wfps[:], qt_sb[:, 0, 0:128],
                                     qt_sb[:, 0, :], start=True, stop=True)

            # ---------------- gathered tiles ------------------------------
            with (
                tc.tile_pool(name="ktg", bufs=16) as ktg_pool,
                tc.tile_pool(name="vg", bufs=1) as vg_pool,
                tc.tile_pool(name="expp", bufs=1) as expp,
            ):
                expS = expp.tile([128, JT, ROWS], BF16)

                # V gathered: per-block tiles on the gpsimd (SWDGE) queues.
                v1_sb = [vg_pool.tile([128, 3, DV], BF16, name=f"v1g{b}")
                         for b in range(NCORES)]
                v2_sb = [vg_pool.tile([128, DV], BF16, name=f"v2g{b}")
                         for b in range(NCORES)]

                # ---------------- S phase --------------------------------
                with (
                    tc.tile_pool(name="ps_s", bufs=2, space="PSUM") as ps_s,
                    tc.tile_pool(name="ps_rs", bufs=1, space="PSUM") as ps_rs,
                ):
                    rs_ps = ps_rs.tile([128, 512], F32)
                    n_rs = 0
                    for half, kt_out_h in ((0, kt1_out), (1, kt2_out)):
                        for b in range(NCORES):
                            ktb = ktg_pool.tile([128, MT, HN], F16, tag="ktg")
                            src = kt_out_h[b * KT_H:(b + 1) * KT_H].rearrange(
                                "(m p n) -> p m n", p=128, n=HN).bitcast(F16)
                            # alternate queues: CC traffic slows concurrent
                            # DRAM reads, so halve each queue's serial depth.
                            # Pairs (0,1),(4,5) on sync and (2,3),(6,7) on
                            # gpsimd so the first four blocks land fastest.
                            eng = nc.sync if b % 4 < 2 else nc.gpsimd
                            eng.dma_start(ktb[:, 0:4, :], src[:, 0:4, :])
                            eng.dma_start(ktb[:, 4:8, :], src[:, 4:8, :])
                            for jj in range(2):
                                j = b * ST + half * 2 + jj
                                ps_S = ps_s.tile([128, 512], F32, tag="pss")
                                for t in range(MT):
                                    nc.tensor.matmul(
                                        ps_S[:],
                                        ktb[:, t, jj * 128:(jj + 1) * 128],
                                        qt_sb[:, t, :],
                                        start=(t == 0), stop=(t == MT - 1))
                                nc.scalar.activation(expS[:, j, :], ps_S[:],
                                                     Exp, bias=bias_sb[:],
                                                     scale=1.0)
                                nc.tensor.matmul(rs_ps[:], ones128[:],
                                                 expS[:, j, :],
                                                 start=(n_rs == 0),
                                                 stop=(n_rs == JT - 1))
                                n_rs += 1

                    # V loads: issued after the S-phase ktb loads so the
                    # gpsimd queue drains kt halves first.
                    for b in range(NCORES):
                        src = v1_out[b * V1_ELEMS:(b + 1) * V1_ELEMS].rearrange(
                            "(s p m) -> p s m", p=128, m=DV)
                        nc.gpsimd.dma_start(v1_sb[b][:], src)
                    for b in range(NCORES):
                        src = v2_out[b * V2_ELEMS:(b + 1) * V2_ELEMS].rearrange(
                            "(s p m) -> p s m", p=128, m=DV)
                        nc.gpsimd.dma_start(v2_sb[b][:], src[:, 0, :])

                    # row-sum -> per-row reciprocal multipliers [128, ST]
                    rs_sb = misc.tile([1, 512], F32)
                    nc.vector.tensor_copy(rs_sb[:], rs_ps[0:1, :])
                    rs_dram = dram.tile([1, 512], F32)
                    nc.sync.dma_start(rs_dram[:], rs_sb[:])
                    rs128 = misc.tile([128, ST], F32)
                    nc.sync.dma_start(
                        rs128[:], rs_dram[0, :].rearrange("(r p) -> p r", p=128))
                    mult_sb = misc.tile([128, ST], F32)
                    nc.vector.reciprocal(mult_sb[:], rs128[:])
                    nc.vector.tensor_scalar_mul(mult_sb[:], mult_sb[:], SCALE)

                # ---------------- AV phase -------------------------------
                # j-outer over all 8 (r,h) PSUM banks: each V block is
                # consumed as soon as it lands; V1 blocks first.
                with tc.tile_pool(name="ps_o", bufs=1, space="PSUM") as ps_o:
                    po = [ps_o.tile([128, 512], F32, name=f"po{i}")
                          for i in range(8)]
                    jorder = ([(b, s) for b in range(NCORES) for s in range(3)]
                              + [(b, 3) for b in range(NCORES)])
                    for idx, (b, s) in enumerate(jorder):
                        j = b * ST + s
                        for r in range(ST):
                            for h in range(2):
                                hs = slice(h * 512, (h + 1) * 512)
                                vap = (v1_sb[b][:, s, hs] if s < 3
                                       else v2_sb[b][:, hs])
                                nc.tensor.matmul(
                                    po[r * 2 + h][:],
                                    expS[:, j, r * 128:(r + 1) * 128],
                                    vap,
                                    start=(idx == 0), stop=(idx == JT - 1))
                    for r in range(ST):
                        for h in range(2):
                            o_sb = outp.tile([128, 512], F32, tag="osb")
                            if h == 0:
                                nc.vector.tensor_scalar_mul(
                                    o_sb[:], po[r * 2 + h][:],
                                    mult_sb[:, r:r + 1])
                            else:
                                nc.scalar.activation(
                                    o_sb[:], po[r * 2 + h][:], Copy,
                                    scale=mult_sb[:, r:r + 1])
                            eng = nc.sync if h == 0 else nc.scalar
                            eng.dma_start(
                                d_out[r * 128:(r + 1) * 128,
                                      h * 512:(h + 1) * 512],
                                o_sb[:])
    nc.compile()
    return nc


_BUILT = None


def kernel(z, Wq, Wk, Wv):
    global _BUILT
    from concourse.bass_utils import run_bass_kernel_spmd

    if _BUILT is None:
        _BUILT = _build()
    nc = _BUILT

    zT = np.ascontiguousarray(z.T).astype(np.float16)
    wq16 = Wq.astype(np.float16)
    wk16 = Wk.astype(np.float16)
    wv16 = Wv.astype(np.float16)
    in_maps = [
        {
            "zT": np.ascontiguousarray(zT[:, c * ROWS:(c + 1) * ROWS]),
            "Wq": wq16,
            "Wk": wk16,
            "Wv": wv16,
        }
        for c in range(NCORES)
    ]
    res = run_bass_kernel_spmd(nc, in_maps, list(range(NCORES)))
    out = np.concatenate([res.results[c]["out"] for c in range(NCORES)], axis=0)
    return out.astype(np.float32)


if __name__ == "__main__":
    rng = np.random.default_rng(0)
    z = rng.standard_normal((SEQ, D)).astype(np.float32)
    Wq = (0.02 * rng.standard_normal((D, DK))).astype(np.float32)
    Wk = (0.02 * rng.standard_normal((D, DK))).astype(np.float32)
    Wv = (0.02 * rng.standard_normal((D, DV))).astype(np.float32)
    out = kernel(z=z, Wq=Wq, Wk=Wk, Wv=Wv)
    print(out.shape, out.dtype)
